# revision 9
# baseline (speedup 1.0000x reference)
"""Multi-head attention Trainium2 kernel, 8-core SPMD.

Problem: x[2,4096,512], 8 heads of 64; per-head QKV proj, softmax(QK^T/8)V,
concat, output proj.

Sharding: sequence-parallel, no collectives. Core c handles batch b=c//4 and
query rows [1024*(c%4), 1024*(c%4)+1024). Each core computes K/V for the full
4096-row sequence of its batch (4x duplicated work, hidden under the ACT exp
bottleneck) and writes its own 1024x512 output slice.

Layouts (SBUF, partition dim first):
  xT   [128,4,512]   x^T chunk: partition=d%128, dsub=d//128, free=t_local
  kT   [128,4,4096]  bf16 K^T: partition p,group g -> row g*128+p = h*64+e
  qT   [128,4,1024]  bf16 Q^T, same row packing, local q cols
  v    [128,32,8,65] bf16 V augmented: [t%128, t//128, h, e(+ones col 64)]
  yT   [128,4,1024]  fp32 attention out^T, rows (h,e), local q cols
Scores are computed transposed (S^T[t,s]) so softmax needs no transposes:
exp on ACT reads score PSUM directly; the ones-column of V makes row 64 of
the PV accumulation equal the softmax denominator.
"""

import numpy as np

import concourse.bass as bass
from concourse import bacc
import concourse.mybir as mybir
import concourse.tile as tile
from concourse.bass_utils import run_bass_kernel_spmd

F32 = mybir.dt.float32
F32R = mybir.dt.float32r
BF16 = mybir.dt.bfloat16

B, S, D, H, E = 2, 4096, 512, 8, 64
NCORES = 8
QCHUNK = S // 4          # 1024 query rows per core
TCH = 512                # t-rows per phase-1 chunk
G = 3                    # score psum banks per exp instruction


def build_program():
    nc = bacc.Bacc()
    xt_d = nc.dram_tensor("xt", [D, S], F32R, kind="ExternalInput")
    wq_d = nc.dram_tensor("wq", [128, 4, 512], F32R, kind="ExternalInput")
    wk_d = nc.dram_tensor("wk", [128, 4, 512], F32R, kind="ExternalInput")
    wv_d = nc.dram_tensor("wv", [128, 4, 512], F32R, kind="ExternalInput")
    wo_d = nc.dram_tensor("wo", [128, 4, 512], F32R, kind="ExternalInput")
    bq_d = nc.dram_tensor("bq", [128, 4], F32, kind="ExternalInput")
    bk_d = nc.dram_tensor("bk", [128, 4], F32, kind="ExternalInput")
    bv_d = nc.dram_tensor("bv", [512], F32, kind="ExternalInput")
    bo_d = nc.dram_tensor("bo", [512], F32, kind="ExternalInput")
    out_d = nc.dram_tensor("out", [QCHUNK, D], F32, kind="ExternalOutput")

    # q0 is passed per-core but we cannot branch on it cheaply; instead each
    # core gets its own x already rolled so its query rows sit at rows 0:1024.
    # (host side rolls x; kernel always uses rows 0:1024 as queries)

    with tile.TileContext(nc) as tc:
        with (
            tc.tile_pool(name="const", bufs=1) as cpool,
            tc.tile_pool(name="work", bufs=2) as wpool,
            tc.tile_pool(name="pt", bufs=3) as ptpool,
            tc.tile_pool(name="ps", bufs=2, space="PSUM") as pspool,
            tc.tile_pool(name="dr", bufs=2, space="DRAM") as dpool,
        ):
            wq_s = cpool.tile([128, 4, 512], F32R, tag="wq")
            wk_s = cpool.tile([128, 4, 512], F32R, tag="wk")
            wv_s = cpool.tile([128, 4, 512], F32R, tag="wv")
            wo_s = cpool.tile([128, 4, 512], F32R, tag="wo")
            bq_s = cpool.tile([128, 4], F32, tag="bq")
            bk_s = cpool.tile([128, 4], F32, tag="bk")
            bv_r = cpool.tile([128, 512], F32, tag="bvr")
            bo_r = cpool.tile([128, 512], F32, tag="bor")
            nc.sync.dma_start(wq_s[:], wq_d[:])
            nc.sync.dma_start(wk_s[:], wk_d[:])
            nc.sync.dma_start(wv_s[:], wv_d[:])
            nc.sync.dma_start(wo_s[:], wo_d[:])
            nc.sync.dma_start(bq_s[:], bq_d[:])
            nc.sync.dma_start(bk_s[:], bk_d[:])
            nc.sync.dma_start(bv_r[:], bv_d[:].unsqueeze(0).to_broadcast((128, 512)))
            nc.sync.dma_start(bo_r[:], bo_d[:].unsqueeze(0).to_broadcast((128, 512)))

            kT = cpool.tile([128, 4, S], BF16, tag="kT")
            qT = cpool.tile([128, 4, QCHUNK], BF16, tag="qT")
            vA = cpool.tile([128, S // 128, H, E + 1], BF16, tag="vA")
            yT = cpool.tile([128, 4, QCHUNK], F32R, tag="yT")
            nc.vector.memset(vA[:, :, :, E], 1.0)

            # ---- phase 1: x -> xT chunks -> K^T, V, Q^T projections ----
            for ch in range(S // TCH):
                xT = wpool.tile([128, 4, TCH], F32R, tag="xT")
                for ds_ in range(4):
                    nc.sync.dma_start(
                        xT[:, ds_, :],
                        xt_d[ds_ * 128 : (ds_ + 1) * 128, ch * TCH : (ch + 1) * TCH],
                    )
                # K^T rows: 4 groups of 128
                for eg in range(4):
                    pk = pspool.tile([128, 512], F32, tag="small")
                    for ds_ in range(4):
                        nc.tensor.matmul(
                            pk[:, :TCH],
                            wk_s[:, ds_, eg * 128 : (eg + 1) * 128],
                            xT[:, ds_, :],
                            start=(ds_ == 0),
                            stop=(ds_ == 3),
                        )
                    nc.vector.tensor_tensor(
                        out=kT[:, eg, ch * TCH : (ch + 1) * TCH],
                        in0=pk[:, :TCH],
                        in1=bk_s[:, eg, None].to_broadcast([128, TCH]),
                        op=mybir.AluOpType.add,
                    )
                # V rows (t on partitions)
                for ts_ in range(TCH // 128):
                    pv = pspool.tile([128, 512], F32, tag="small")
                    for ds_ in range(4):
                        nc.tensor.matmul(
                            pv[:],
                            xT[:, ds_, ts_ * 128 : (ts_ + 1) * 128],
                            wv_s[:, ds_, :],
                            start=(ds_ == 0),
                            stop=(ds_ == 3),
                        )
                    nc.vector.tensor_tensor(
                        out=vA[:, ch * 4 + ts_, :, 0:E],
                        in0=pv[:].rearrange("p (h e) -> p h e", h=H),
                        in1=bv_r[:].rearrange("p (h e) -> p h e", h=H),
                        op=mybir.AluOpType.add,
                    )
                # Q^T for query chunks (local rows 0:1024 of this core's x)
                if ch < QCHUNK // TCH:
                    for eg in range(4):
                        pq = pspool.tile([128, 512], F32, tag="small")
                        for ds_ in range(4):
                            nc.tensor.matmul(
                                pq[:, :TCH],
                                wq_s[:, ds_, eg * 128 : (eg + 1) * 128],
                                xT[:, ds_, :],
                                start=(ds_ == 0),
                                stop=(ds_ == 3),
                            )
                        nc.vector.tensor_tensor(
                            out=qT[:, eg, ch * TCH : (ch + 1) * TCH],
                            in0=pq[:, :TCH],
                            in1=bq_s[:, eg, None].to_broadcast([128, TCH]),
                            op=mybir.AluOpType.add,
                        )

            # ---- phase 2: attention per head / 512-wide query chunk ----
            NT = S // 128          # 32 t-tiles
            for hp in range(H // 2):
                g = hp
                for sc in range(QCHUNK // 512):
                    pav0 = pspool.tile([128, 512], F32, tag="av")
                    pav1 = pspool.tile([128, 512], F32, tag="av")
                    for tt in range(NT):
                        psc = pspool.tile([128, 2, 512], F32, tag="sc")
                        for hh in range(2):
                            p0 = hh * 64
                            nc.tensor.matmul(
                                psc[:, hh, :],
                                kT[p0 : p0 + 64, g, tt * 128 : (tt + 1) * 128],
                                qT[p0 : p0 + 64, g, sc * 512 : (sc + 1) * 512],
                                start=True,
                                stop=True,
                            )
                        pt = ptpool.tile([128, 2, 512], BF16, tag="pt")
                        nc.scalar.activation(
                            pt[:],
                            psc[:],
                            mybir.ActivationFunctionType.Exp,
                            scale=0.125,
                        )
                        for hh, pav in ((0, pav0), (1, pav1)):
                            nc.tensor.matmul(
                                pav[0:65, :],
                                vA[:, tt, 2 * hp + hh, :],
                                pt[:, hh, :],
                                start=(tt == 0),
                                stop=(tt == NT - 1),
                            )
                    for hh, pav in ((0, pav0), (1, pav1)):
                        p0 = hh * 64
                        rec = wpool.tile([1, 512], F32, tag="rec")
                        nc.vector.reciprocal(rec[:], pav[64:65, :])
                        rrep = wpool.tile([64, 512], F32, tag="rrep")
                        rec_d = dpool.tile([1, 512], F32, tag="recd")
                        nc.sync.dma_start(rec_d[:], rec[:])
                        nc.sync.dma_start(rrep[:], rec_d[:].to_broadcast((64, 512)))
                        nc.vector.tensor_tensor(
                            out=yT[p0 : p0 + 64, g, sc * 512 : (sc + 1) * 512],
                            in0=pav[0:64, :],
                            in1=rrep[:],
                            op=mybir.AluOpType.mult,
                        )

            # ---- phase 3: output projection ----
            for st in range(QCHUNK // 128):
                po = pspool.tile([128, 512], F32, tag="small")
                for g in range(4):
                    nc.tensor.matmul(
                        po[:],
                        yT[:, g, st * 128 : (st + 1) * 128],
                        wo_s[:, g, :],
                        start=(g == 0),
                        stop=(g == 3),
                    )
                o_s = wpool.tile([128, 512], F32, tag="osb")
                nc.vector.tensor_tensor(o_s[:], po[:], bo_r[:], mybir.AluOpType.add
                )
                nc.sync.dma_start(
                    out_d[st * 128 : (st + 1) * 128, :], o_s[:]
                )
    nc.compile()
    return nc


_NC = None


def kernel(x, Wq, bq, Wk, bk, Wv, bv, Wo, bo, **kw):
    global _NC
    x = np.asarray(x, np.float32)
    s = lambda a: np.ascontiguousarray(np.asarray(a, np.float32))
    # weight packing shared by all cores
    wq_p = s(np.transpose(Wq, (1, 0, 2)).reshape(D, 512).reshape(4, 128, 512)
             .transpose(1, 0, 2))
    wk_p = s(np.transpose(Wk, (1, 0, 2)).reshape(D, 512).reshape(4, 128, 512)
             .transpose(1, 0, 2))
    wv_p = s(np.transpose(Wv, (1, 0, 2)).reshape(D, 512).reshape(4, 128, 512)
             .transpose(1, 0, 2))
    wo_p = s(np.asarray(Wo, np.float32).reshape(4, 128, 512).transpose(1, 0, 2))
    bq_p = s(np.asarray(bq, np.float32).reshape(512).reshape(4, 128).T)
    bk_p = s(np.asarray(bk, np.float32).reshape(512).reshape(4, 128).T)
    bv_p = s(np.asarray(bv, np.float32).reshape(512))
    bo_p = s(np.asarray(bo, np.float32))

    if _NC is None:
        _NC = build_program()

    in_maps = []
    for c in range(NCORES):
        b = c // 4
        q0 = (c % 4) * QCHUNK
        xb = np.roll(x[b], -q0, axis=0)  # queries at rows 0:1024
        in_maps.append({
            "xt": np.ascontiguousarray(xb.T),
            "wq": wq_p, "wk": wk_p, "wv": wv_p, "wo": wo_p,
            "bq": bq_p, "bk": bk_p, "bv": bv_p, "bo": bo_p,
        })
    res = run_bass_kernel_spmd(_NC, in_maps, core_ids=list(range(NCORES)))
    out = np.empty((B, S, D), np.float32)
    for c in range(NCORES):
        b = c // 4
        q0 = (c % 4) * QCHUNK
        out[b, q0 : q0 + QCHUNK] = res.results[c]["out"]
    return out
